# revision 77
# baseline (speedup 1.0000x reference)
"""Multi-head causal attention (B=2, L=2048, H=2048, NH=16) on 8 Trainium2
NeuronCores.

Sharding: tensor-parallel over heads — core c computes heads {2c, 2c+1}.
Host sums the 8 partial o-projection outputs.

Design (vs the fp32r phase-split baseline, 418us -> ~343us):
  - all matmul operands in fp16: same PE rate as fp32r (1 cyc/row, and
    no 256-col minimum), half the HBM traffic and SBUF footprint, and
    an 11-bit mantissa that keeps rel err ~5e-4
  - phase-1 (q/k/v proj) and phase-2 (attention) FUSED per 512-token
    chunk: causal attention for chunk g needs only k/v of chunks <= g,
    so the PE stream never crosses an idle phase boundary (TRN2 drops
    to 1.2 GHz for ~3us after any PE idle gap)
  - softmax denominator: DVE accumulates exp tiles into an fp16 acc
    (16-bit DVE runs 2x), the last two tiles skip the acc and feed a
    short PSUM-accumulated ones-matmul rowsum directly, so the PE
    never waits on the DVE queue; then ACT row-copy (releases the rs
    bank fast), DVE reciprocal, gpsimd broadcast, DVE normalize
  - causal mask via gpsimd affine_select off the PE critical path
    (DVE tensor_mul with a DRAM-loaded triangular tile for chunk 0,
    while gpsimd is busy generating startup SWDGE descriptors);
    diagonal j-tiles narrowed to their exact live width
  - score matmuls run 3 j-tiles ahead of attnV, and the attnV order
    delays each diagonal tile one further slot than its score, so the
    exp -> gpsimd-mask chain always has ~2us of PE cover; the previous
    chunk's o-projection is spread as PE filler through the attention
    loop, with 2 fillers reserved to cover each rowsum's last DVE add
  - o-proj outputs cast to fp16 (DVE/ACT alternating), 4 o-tiles per
    DMA on the sync queue; the final flush uses 2-ot groups rotated
    across three queues so the drain overlaps; host sums in fp32
  - dedicated 2-bank PSUM pool for the attnV accumulators so bank
    recycling never chains phase-1 onto the softmax-normalize tail

Measured (neuron-profile, core 0): ~343us, rel err absmax ~5.6e-4.
PE streaming floor for this schedule is ~284us; the rest is NEFF
pre/post overhead (~12us), residual DVE-latency stalls, and the
1.2 GHz p-state windows they cause.  fp8 DoubleRow hi/lo projections
were tried and are a net loss on HW (DoubleRow streams ~2.6x slower
than the cost model's 0.5 cyc/row, and LDWEIGHTS doubles).
"""

import os
import sys

if "/opt/trn_rl_repo" not in sys.path:
    sys.path.insert(0, "/opt/trn_rl_repo")

import numpy as np
import ml_dtypes

from concourse import bacc, mybir, tile  # noqa: E402
from concourse.bass_utils import run_bass_kernel_spmd  # noqa: E402

F16 = mybir.dt.float16
F32 = mybir.dt.float32

N_CORES = 8
B, L, H, NH = 2, 2048, 2048, 16
DH = H // NH                       # 128
BL = B * L                        # 4096
HPC = NH // N_CORES               # heads per core = 2
OPC = HPC * DH                    # output dims per core = 256
HT = H // 128                     # 16 h-tiles (contraction)
IC = 512                          # chunk width (tokens)
NCH = BL // IC                    # 8 global chunks (4 per batch)
JTB = L // 128                    # 16 j-tiles per batch
SCALE = 1.0 / float(np.sqrt(DH))

LAST_EXEC_NS = None


def _build():
    nc = bacc.Bacc(None, target_bir_lowering=False, debug=False)

    xt = nc.declare_dram_parameter("xt", [H, BL], F16, isOutput=False)
    wq = nc.declare_dram_parameter("wq", [H, OPC], F16, isOutput=False)
    wk = nc.declare_dram_parameter("wk", [H, OPC], F16, isOutput=False)
    wv = nc.declare_dram_parameter("wv", [H, OPC], F16, isOutput=False)
    wo = nc.declare_dram_parameter("wo", [OPC, H], F16, isOutput=False)
    tri = nc.declare_dram_parameter("tri", [128, IC], F16, isOutput=False)
    out = nc.declare_dram_parameter("out", [H, BL], F16, isOutput=True)
    # last chunk's h1-only o-proj partial (host adds it into out's
    # final column block) so the tail only serializes half the flush
    out2 = nc.declare_dram_parameter("out2", [H, IC], F16, isOutput=True)

    with tile.TileContext(nc) as tc:
        with tc.tile_pool(name="persist", bufs=1) as persist, \
             tc.tile_pool(name="psum", bufs=6, space="PSUM") as psum, \
             tc.tile_pool(name="mx_psum", bufs=2, space="PSUM") as mx_psum, \
             tc.tile_pool(name="xpool", bufs=2) as xpool, \
             tc.tile_pool(name="qt_pool", bufs=2) as qt_pool, \
             tc.tile_pool(name="exp_pool", bufs=8) as exp_pool, \
             tc.tile_pool(name="acc_pool", bufs=2) as acc_pool, \
             tc.tile_pool(name="rr_pool", bufs=2) as rr_pool, \
             tc.tile_pool(name="mst_pool", bufs=3) as mst_pool, \
             tc.tile_pool(name="oc_pool", bufs=4) as oc_pool:
            kt_sb = persist.tile([128, HPC, BL], F16, tag="kt")
            v_sb = persist.tile([128, BL // 128, OPC], F16, tag="v")
            wq_sb = persist.tile([128, HT, OPC], F16, tag="wq")
            wk_sb = persist.tile([128, HT, OPC], F16, tag="wk")
            wv_sb = persist.tile([128, HT, OPC], F16, tag="wv")
            wo_sb = persist.tile([128, HPC, H], F16, tag="wo")

            def dma_split(dst3, src2d, pieces=2):
                # split a [128, T, F]-tile transfer across both HW queues;
                # finer pieces let dependent matmuls start sooner
                t = dst3.shape[1]
                r = src2d.rearrange("(q t p) f -> q p t f", q=pieces, p=128)
                step = t // pieces
                for i in range(pieces):
                    eng = nc.sync if i % 2 == 0 else nc.scalar
                    eng.dma_start(out=dst3[:, i * step:(i + 1) * step, :],
                                  in_=r[i])

            # startup DMAs: wq in 4-ht pieces on the gpsimd SWDGE queue
            # so the two HWDGE queues spend the ramp on x/wk/wv (few
            # pieces — SWDGE generation costs ~1us each on gpsimd)
            wq_r = wq[:, :].rearrange("(q t p) f -> q p t f", q=4, p=128)
            for qp in range(4):
                nc.gpsimd.dma_start(
                    out=wq_sb[:, qp * 4:(qp + 1) * 4, :], in_=wq_r[qp])
            tri_sb = persist.tile([128, IC], F16, tag="tri")
            xchs = {}
            xchs[0] = xpool.tile([128, HT, IC], F16, tag="xch", name="xch")
            dma_split(xchs[0], xt[:, 0:IC], pieces=4)
            nc.scalar.dma_start(out=tri_sb[:, :], in_=tri[:, :])
            dma_split(wk_sb, wk[:, :])
            dma_split(wv_sb, wv[:, :])
            dma_split(wo_sb, wo[:, :])

            ones_sb = persist.tile([128, 1], F16, tag="ones")
            ones_f = rr_pool.tile([128, IC], F32, tag="red", name="onesf")
            nc.vector.memset(ones_f[:, 0:1], 1.0)
            nc.vector.tensor_copy(ones_sb[:, :], ones_f[:, 0:1])

            # PE pre-warm: dummy matmuls on a memset tile (no DMA
            # dependency) burn the 1.2 GHz p-state ramp during the
            # startup DMA wait instead of during the first real chains
            warm = exp_pool.tile([128, IC], F16, tag="ex", name="warm")
            nc.vector.memset(warm[:, :], 0.0)
            wps = psum.tile([128, IC], F32, tag="bank", name="wps")
            for _ in range(10):
                nc.tensor.matmul(wps[:, :], warm[:, 0:128], warm[:, :],
                                 start=True, stop=True)

            # ---- o-projection emitter (for the chunk before `gio`) ----
            oc_state = {"oc": None}

            def emit_oproj_single(mst, ot, hh, dst_dram, dcol):
                op = psum.tile([128, IC], F32, tag="bank", name="op")
                nc.tensor.matmul(
                    op[:, :], wo_sb[:, hh, ot * 128:(ot + 1) * 128],
                    mst[:, hh, :], start=True, stop=True)
                if ot % 4 == 0:
                    oc_state["oc1"] = oc_pool.tile([128, 4, IC], F16,
                                                   tag="oc", name="oc1")
                oc = oc_state["oc1"]
                if ot % 2 == 1:
                    nc.scalar.copy(oc[:, ot % 4, :], op[:, :])
                else:
                    nc.vector.tensor_copy(oc[:, ot % 4, :], op[:, :])
                if ot % 4 == 3:
                    dst = dst_dram[(ot - 3) * 128:(ot + 1) * 128,
                                   dcol:dcol + IC]
                    deng = (nc.sync, nc.scalar)[(ot // 4) % 2]
                    deng.dma_start(
                        out=dst.rearrange("(t p) f -> p t f", p=128),
                        in_=oc[:, :, :])

            def emit_oproj_ot(mst, gio, ot):
                op = psum.tile([128, IC], F32, tag="bank", name="op")
                for hh in range(HPC):
                    nc.tensor.matmul(
                        op[:, :],
                        wo_sb[:, hh, ot * 128:(ot + 1) * 128],
                        mst[:, hh, :],
                        start=(hh == 0), stop=(hh == HPC - 1))
                if ot % 4 == 0:
                    oc_state["oc"] = oc_pool.tile([128, 4, IC], F16,
                                                  tag="oc", name="oc")
                oc = oc_state["oc"]
                if ot % 2 == 1 or oc_state.get("act_only"):
                    nc.scalar.copy(oc[:, ot % 4, :], op[:, :])
                else:
                    nc.vector.tensor_copy(oc[:, ot % 4, :], op[:, :])
                if ot % 4 == 3:
                    # sync only: a gpsimd SWDGE here would delay the
                    # latency-critical affine_select masks
                    dst = out[(ot - 3) * 128:(ot + 1) * 128, gio:gio + IC]
                    nc.sync.dma_start(
                        out=dst.rearrange("(t p) f -> p t f", p=128),
                        in_=oc[:, :, :])

            pending = None
            for g in range(NCH):
                b, ic = divmod(g, NCH // B)
                gio = g * IC
                # ---------------- phase 1: q/k/v for chunk g ----------------
                if g + 1 < NCH:
                    xchs[g + 1] = xpool.tile([128, HT, IC], F16,
                                             tag="xch", name="xch")
                    dma_split(xchs[g + 1], xt[:, (g + 1) * IC:(g + 2) * IC],
                              pieces=(4 if g == 0 else 2))
                xch = xchs.pop(g)
                qt_g = qt_pool.tile([128, HPC, IC], F16, tag="qt", name="qt")
                for wsb, dst in ((wq_sb, None), (wk_sb, kt_sb)):
                    for ot in range(HPC):
                        ps = psum.tile([128, IC], F32, tag="bank", name="ps")
                        for ht in range(HT):
                            nc.tensor.matmul(
                                ps[:, :],
                                wsb[:, ht, ot * 128:(ot + 1) * 128],
                                xch[:, ht, :],
                                start=(ht == 0), stop=(ht == HT - 1))
                        dst_ap = (qt_g[:, ot, :] if dst is None
                                  else dst[:, ot, gio:gio + IC])
                        # q/k copies stay off the laggy DVE queue: the
                        # same-chunk score matmuls depend on them
                        nc.scalar.copy(dst_ap, ps[:, :])
                # ---------------- phase 2: attention for chunk g ------------
                njt = 4 * ic + 4
                # full-width tile first (initializes acc/mx); diagonal
                # tiles (whose exp->mask chain is longest) interleaved
                # among full-width tiles to hide the gpsimd mask latency
                diags = list(range(max(1, 4 * ic), 4 * ic + 4))
                fulls = list(range(1, 4 * ic))
                # scores: diagonals early (their exp->mask chain is the
                # longest); attnV: each diagonal delayed one more slot so
                # the gpsimd mask has ~2 extra PE iterations of cover
                order = [0]
                for i, d in enumerate(diags):
                    order.append(d)
                    if i < len(fulls):
                        order.append(fulls[i])
                order += fulls[len(diags):]
                av_order = [0]
                for i, d in enumerate(diags):
                    if i < len(fulls):
                        av_order.append(fulls[i])
                    av_order.append(d)
                av_order += fulls[len(diags):]

                exds = [{}, {}]

                def emit_sc_exp(h, jt):
                    f0 = max(0, 128 * jt - IC * ic)
                    w = IC - f0
                    sc = psum.tile([128, IC], F32, tag="bank", name="sc")
                    nc.tensor.matmul(
                        sc[:, f0:],
                        kt_sb[:, h, b * L + jt * 128:
                              b * L + (jt + 1) * 128],
                        qt_g[:, h, f0:],
                        start=True, stop=True)
                    ex = exp_pool.tile([128, IC], F16, tag="ex")
                    nc.scalar.activation(
                        ex[:, f0:], sc[:, f0:],
                        mybir.ActivationFunctionType.Exp,
                        scale=SCALE)
                    if jt >= 4 * ic:
                        # zero where local j-partition > local i-column.
                        # gpsimd affine_select: slower as an op (~1.1us)
                        # than a DVE tri-multiply (~0.25us), but the DVE
                        # queue lag makes gpsimd faster end-to-end; DVE
                        # only for chunk 0 while gpsimd generates the
                        # startup SWDGE descriptors.
                        if g == 0:
                            nc.vector.tensor_mul(
                                ex[:, f0:], ex[:, f0:], tri_sb[:, :w])
                        else:
                            nc.gpsimd.affine_select(
                                ex[:, f0:], ex[:, f0:],
                                pattern=[[1, w]],
                                compare_op=mybir.AluOpType.is_ge,
                                fill=0.0, base=0, channel_multiplier=-1)
                    exds[h][jt] = (ex, f0)

                # h0 warmup before the v-projection chains: exp gets a
                # whole projection pass of ACT lead time
                for w0 in range(min(3, njt)):
                    emit_sc_exp(0, order[w0])

                # v in natural (j x o) layout, two 128-token chains per bank
                for half in range(2):
                    vps = psum.tile([128, 2, OPC], F32, tag="bank", name="ps")
                    for sub in range(2):
                        it = half * 2 + sub
                        for ht in range(HT):
                            nc.tensor.matmul(
                                vps[:, sub, :],
                                xch[:, ht, it * 128:(it + 1) * 128],
                                wv_sb[:, ht, :],
                                start=(ht == 0), stop=(ht == HT - 1))
                    vdst = v_sb[:, g * 4 + half * 2:g * 4 + half * 2 + 2, :]
                    # v feeds this chunk's diagonal attnVs — keep both
                    # halves off the laggy DVE queue
                    nc.scalar.copy(vdst, vps[:, :, :])
                filler = []
                if pending is not None:
                    pmst, pgio = pending
                    filler = [(pmst, pgio, ot) for ot in range(H // 128)]
                fill_idx = 0
                # keep the closing DVE chain of the last chunk unqueued
                oc_state["act_only"] = (g == NCH - 1)
                total_iters = HPC * njt
                it_counter = 0
                for h in range(HPC):
                    mx = mx_psum.tile([128, IC], F32, tag="mx", name="mx")
                    # fp16 accumulator: feeds the PE rowsum directly
                    # (no cast), DVE 16-bit ops run at the 2x rate
                    acc = acc_pool.tile([128, IC], F16, tag="acc", name="acc")
                    exd = exds[h]

                    if h == 1:
                        for w0 in range(min(3, njt)):
                            emit_sc_exp(h, order[w0])
                    # last two tiles skip the DVE acc and feed the PE
                    # rowsum directly, so the rowsum never waits on DVE
                    direct = []
                    for idx in range(njt):
                        jt = av_order[idx]
                        it_counter += 1
                        if idx + 3 < njt:
                            emit_sc_exp(h, order[idx + 3])
                        ex, f0 = exd.pop(jt)
                        nc.tensor.matmul(
                            mx[:, f0:],
                            v_sb[:, b * JTB + jt, h * 128:(h + 1) * 128],
                            ex[:, f0:],
                            start=(idx == 0), stop=(idx == njt - 1))
                        if idx == 0:
                            nc.vector.tensor_copy(acc[:, :], ex[:, :])
                        elif idx < njt - 2:
                            nc.vector.tensor_add(
                                acc[:, f0:], acc[:, f0:], ex[:, f0:])
                        else:
                            direct.append((ex, f0))
                        # hold 2 fillers back so each h's rowsum below
                        # always has PE cover for the last DVE add
                        want = min((it_counter * len(filler)) // total_iters,
                                   max(0, len(filler) - 2))
                        while fill_idx < want:
                            emit_oproj_ot(*filler[fill_idx])
                            fill_idx += 1
                        if h == 1 and g == NCH - 1 and idx >= 3 \
                                and oc_state["h0fl"]:
                            emit_oproj_single(mst, oc_state["h0fl"].pop(0),
                                              0, out, gio)
                    # fillers before the rowsum cover the last DVE add's
                    # queue lag
                    for _ in range(2):
                        if fill_idx < len(filler):
                            emit_oproj_ot(*filler[fill_idx])
                            fill_idx += 1
                    # inline rowsum: acc + the two direct ex tiles
                    rs = psum.tile([128, IC], F32, tag="bank", name="rs")
                    nc.tensor.matmul(rs[0:1, :], ones_sb[:, :], acc[:, :],
                                     start=True, stop=False)
                    for di, (ex, f0) in enumerate(direct):
                        nc.tensor.matmul(
                            rs[0:1, f0:], ones_sb[:, :], ex[:, f0:],
                            start=False, stop=(di == len(direct) - 1))
                    # ACT row-copy releases the rs bank fast (DVE recip
                    # would hold it ~2us and starve the sc ring)
                    tmp = rr_pool.tile([128, IC], F32, tag="red", name="tmp")
                    nc.scalar.copy(tmp[0:1, :], rs[0:1, :])
                    rec = rr_pool.tile([128, IC], F32, tag="red", name="rec")
                    nc.vector.reciprocal_approx_fast(
                        out=rec[0:1, :], in_=tmp[0:1, :])
                    nc.gpsimd.partition_broadcast(
                        rec[:, :], rec[0:1, :], channels=128)
                    mst = (mst_pool.tile([128, HPC, IC], F16, tag="mst",
                                         name="mst")
                           if h == 0 else mst)
                    nc.vector.tensor_mul(mst[:, h, :], mx[:, :], rec[:, :])
                    if g == NCH - 1 and h == 0:
                        oc_state["h0fl"] = list(range(H // 128))
                while fill_idx < len(filler):
                    emit_oproj_ot(*filler[fill_idx])
                    fill_idx += 1
                pending = (mst, gio)
            # final flush: any leftover h0 groups, then the h1-only
            # half into out2 (host adds it) — the tail after the last
            # normalize serializes only half the o-projection
            pmst, pgio = pending
            while oc_state["h0fl"]:
                emit_oproj_single(pmst, oc_state["h0fl"].pop(0), 0, out,
                                  pgio)
            for ot in range(H // 128):
                op = psum.tile([128, IC], F32, tag="bank", name="op")
                nc.tensor.matmul(
                    op[:, :], wo_sb[:, 1, ot * 128:(ot + 1) * 128],
                    pmst[:, 1, :], start=True, stop=True)
                if ot % 2 == 0:
                    oc_state["oc"] = oc_pool.tile([128, 2, IC], F16,
                                                  tag="oc2", name="oc2")
                oc = oc_state["oc"]
                if ot % 2 == 1:
                    nc.scalar.copy(oc[:, ot % 2, :], op[:, :])
                else:
                    nc.vector.tensor_copy(oc[:, ot % 2, :], op[:, :])
                if ot % 2 == 1:
                    deng = (nc.sync, nc.scalar, nc.gpsimd)[(ot // 2) % 3]
                    dst = out2[(ot - 1) * 128:(ot + 1) * 128, 0:IC]
                    deng.dma_start(
                        out=dst.rearrange("(t p) f -> p t f", p=128),
                        in_=oc[:, :, :])
    nc.finalize()
    return nc


_NC_CACHE = None


def _get_nc():
    global _NC_CACHE
    if _NC_CACHE is None:
        _NC_CACHE = _build()
    return _NC_CACHE


def _install_hook_shim():
    """Make antenv.axon_hooks importable (absent on this image) so
    run_bass_kernel_spmd's trace path degrades gracefully."""
    import types
    import antenv
    if "antenv.axon_hooks" not in sys.modules:
        shim = types.ModuleType("antenv.axon_hooks")

        def set_axon_ntff_profile_hook(h):
            shim._the_hook = h

        def get_axon_ntff_profile_hook():
            return getattr(shim, "_the_hook", None)

        shim.set_axon_ntff_profile_hook = set_axon_ntff_profile_hook
        shim.get_axon_ntff_profile_hook = get_axon_ntff_profile_hook
        sys.modules["antenv.axon_hooks"] = shim
        antenv.axon_hooks = shim


def _enable_profiling():
    """Wire the axon NTFF profile hook for neuron-profile timing."""
    _install_hook_shim()
    from trn_agent_boot.trn_boot import _ntff_profile_via_ctypes
    hook = _ntff_profile_via_ctypes("/opt/axon/libaxon_pjrt.so")
    sys.modules["antenv.axon_hooks"].set_axon_ntff_profile_hook(hook)
    import concourse.bass_utils as bu
    bu.upload_artifacts = lambda tmpdir: "local://" + tmpdir


def kernel(x, padding_mask, Wq, Wk, Wv, Wo):
    global LAST_EXEC_NS
    f16 = np.float16
    x = np.asarray(x, dtype=np.float32)

    xt = np.ascontiguousarray(x.reshape(BL, H).T).astype(f16)   # (H, BL)
    wqt = np.asarray(Wq, dtype=np.float32).T.astype(f16)        # [h, o]
    wkt = np.asarray(Wk, dtype=np.float32).T.astype(f16)
    wvt = np.asarray(Wv, dtype=np.float32).T.astype(f16)
    wot = np.asarray(Wo, dtype=np.float32).T.astype(f16)        # [h_in, o]

    tri = np.triu(np.ones((128, IC), dtype=f16))

    in_maps = []
    for c in range(N_CORES):
        sl = slice(c * OPC, (c + 1) * OPC)
        in_maps.append({
            "xt": xt,
            "wq": np.ascontiguousarray(wqt[:, sl]),
            "wk": np.ascontiguousarray(wkt[:, sl]),
            "wv": np.ascontiguousarray(wvt[:, sl]),
            "wo": np.ascontiguousarray(wot[sl, :]),
            "tri": tri,
        })

    profile = os.environ.get("KERNEL_PROFILE", "0") == "1"
    try:
        if profile:
            _enable_profiling()
        else:
            _install_hook_shim()
    except Exception:
        profile = False

    nc = _get_nc()
    res = run_bass_kernel_spmd(nc, in_maps, core_ids=list(range(N_CORES)),
                               trace=profile)
    LAST_EXEC_NS = res.exec_time_ns

    total = np.zeros((H, BL), dtype=np.float32)
    for c in range(N_CORES):
        total += np.asarray(res.results[c]["out"], dtype=np.float32)
        total[:, BL - IC:] += np.asarray(res.results[c]["out2"],
                                         dtype=np.float32)
    return np.ascontiguousarray(total.T).astype(np.float32).reshape(B, L, H)


# revision 78
# speedup vs baseline: 1.0174x; 1.0174x over previous
"""Multi-head causal attention (B=2, L=2048, H=2048, NH=16) on 8 Trainium2
NeuronCores.

Sharding: tensor-parallel over heads — core c computes heads {2c, 2c+1}.
Host sums the 8 partial o-projection outputs.

Design (vs the fp32r phase-split baseline, 418us -> ~343us):
  - all matmul operands in fp16: same PE rate as fp32r (1 cyc/row, and
    no 256-col minimum), half the HBM traffic and SBUF footprint, and
    an 11-bit mantissa that keeps rel err ~5e-4
  - phase-1 (q/k/v proj) and phase-2 (attention) FUSED per 512-token
    chunk: causal attention for chunk g needs only k/v of chunks <= g,
    so the PE stream never crosses an idle phase boundary (TRN2 drops
    to 1.2 GHz for ~3us after any PE idle gap)
  - softmax denominator: DVE accumulates exp tiles into an fp16 acc
    (16-bit DVE runs 2x), the last two tiles skip the acc and feed a
    short PSUM-accumulated ones-matmul rowsum directly, so the PE
    never waits on the DVE queue; then ACT row-copy (releases the rs
    bank fast), DVE reciprocal, gpsimd broadcast, DVE normalize
  - causal mask via gpsimd affine_select off the PE critical path
    (DVE tensor_mul with a DRAM-loaded triangular tile for chunk 0,
    while gpsimd is busy generating startup SWDGE descriptors);
    diagonal j-tiles narrowed to their exact live width
  - score matmuls run 3 j-tiles ahead of attnV, and the attnV order
    delays each diagonal tile one further slot than its score, so the
    exp -> gpsimd-mask chain always has ~2us of PE cover; the previous
    chunk's o-projection is spread as PE filler through the attention
    loop, with 2 fillers reserved to cover each rowsum's last DVE add
  - o-proj outputs cast to fp16 (DVE/ACT alternating), 4 o-tiles per
    DMA on the sync queue; the final flush uses 2-ot groups rotated
    across three queues so the drain overlaps; host sums in fp32
  - dedicated 2-bank PSUM pool for the attnV accumulators so bank
    recycling never chains phase-1 onto the softmax-normalize tail

Measured (neuron-profile, core 0): ~343us, rel err absmax ~5.6e-4.
PE streaming floor for this schedule is ~284us; the rest is NEFF
pre/post overhead (~12us), residual DVE-latency stalls, and the
1.2 GHz p-state windows they cause.  fp8 DoubleRow hi/lo projections
were tried and are a net loss on HW (DoubleRow streams ~2.6x slower
than the cost model's 0.5 cyc/row, and LDWEIGHTS doubles).
"""

import os
import sys

if "/opt/trn_rl_repo" not in sys.path:
    sys.path.insert(0, "/opt/trn_rl_repo")

import numpy as np
import ml_dtypes

from concourse import bacc, mybir, tile  # noqa: E402
from concourse.bass_utils import run_bass_kernel_spmd  # noqa: E402

F16 = mybir.dt.float16
F32 = mybir.dt.float32

N_CORES = 8
B, L, H, NH = 2, 2048, 2048, 16
DH = H // NH                       # 128
BL = B * L                        # 4096
HPC = NH // N_CORES               # heads per core = 2
OPC = HPC * DH                    # output dims per core = 256
HT = H // 128                     # 16 h-tiles (contraction)
IC = 512                          # chunk width (tokens)
NCH = BL // IC                    # 8 global chunks (4 per batch)
JTB = L // 128                    # 16 j-tiles per batch
SCALE = 1.0 / float(np.sqrt(DH))

LAST_EXEC_NS = None


def _build():
    nc = bacc.Bacc(None, target_bir_lowering=False, debug=False)

    xt = nc.declare_dram_parameter("xt", [H, BL], F16, isOutput=False)
    wq = nc.declare_dram_parameter("wq", [H, OPC], F16, isOutput=False)
    wk = nc.declare_dram_parameter("wk", [H, OPC], F16, isOutput=False)
    wv = nc.declare_dram_parameter("wv", [H, OPC], F16, isOutput=False)
    wo = nc.declare_dram_parameter("wo", [OPC, H], F16, isOutput=False)
    tri = nc.declare_dram_parameter("tri", [128, IC], F16, isOutput=False)
    out = nc.declare_dram_parameter("out", [H, BL], F16, isOutput=True)

    with tile.TileContext(nc) as tc:
        with tc.tile_pool(name="persist", bufs=1) as persist, \
             tc.tile_pool(name="psum", bufs=6, space="PSUM") as psum, \
             tc.tile_pool(name="mx_psum", bufs=2, space="PSUM") as mx_psum, \
             tc.tile_pool(name="xpool", bufs=2) as xpool, \
             tc.tile_pool(name="qt_pool", bufs=2) as qt_pool, \
             tc.tile_pool(name="exp_pool", bufs=8) as exp_pool, \
             tc.tile_pool(name="acc_pool", bufs=2) as acc_pool, \
             tc.tile_pool(name="rr_pool", bufs=2) as rr_pool, \
             tc.tile_pool(name="mst_pool", bufs=3) as mst_pool, \
             tc.tile_pool(name="oc_pool", bufs=4) as oc_pool:
            kt_sb = persist.tile([128, HPC, BL], F16, tag="kt")
            v_sb = persist.tile([128, BL // 128, OPC], F16, tag="v")
            wq_sb = persist.tile([128, HT, OPC], F16, tag="wq")
            wk_sb = persist.tile([128, HT, OPC], F16, tag="wk")
            wv_sb = persist.tile([128, HT, OPC], F16, tag="wv")
            wo_sb = persist.tile([128, HPC, H], F16, tag="wo")

            def dma_split(dst3, src2d, pieces=2):
                # split a [128, T, F]-tile transfer across both HW queues;
                # finer pieces let dependent matmuls start sooner
                t = dst3.shape[1]
                r = src2d.rearrange("(q t p) f -> q p t f", q=pieces, p=128)
                step = t // pieces
                for i in range(pieces):
                    eng = nc.sync if i % 2 == 0 else nc.scalar
                    eng.dma_start(out=dst3[:, i * step:(i + 1) * step, :],
                                  in_=r[i])

            # startup DMAs: wq in 4-ht pieces on the gpsimd SWDGE queue
            # so the two HWDGE queues spend the ramp on x/wk/wv (few
            # pieces — SWDGE generation costs ~1us each on gpsimd)
            wq_r = wq[:, :].rearrange("(q t p) f -> q p t f", q=4, p=128)
            for qp in range(4):
                nc.gpsimd.dma_start(
                    out=wq_sb[:, qp * 4:(qp + 1) * 4, :], in_=wq_r[qp])
            tri_sb = persist.tile([128, IC], F16, tag="tri")
            xchs = {}
            xchs[0] = xpool.tile([128, HT, IC], F16, tag="xch", name="xch")
            dma_split(xchs[0], xt[:, 0:IC], pieces=4)
            nc.scalar.dma_start(out=tri_sb[:, :], in_=tri[:, :])
            dma_split(wk_sb, wk[:, :])
            dma_split(wv_sb, wv[:, :])
            dma_split(wo_sb, wo[:, :])

            ones_sb = persist.tile([128, 1], F16, tag="ones")
            ones_f = rr_pool.tile([128, IC], F32, tag="red", name="onesf")
            nc.vector.memset(ones_f[:, 0:1], 1.0)
            nc.vector.tensor_copy(ones_sb[:, :], ones_f[:, 0:1])

            # ---- o-projection emitter (for the chunk before `gio`) ----
            oc_state = {"oc": None}

            def emit_oproj_ot(mst, gio, ot):
                op = psum.tile([128, IC], F32, tag="bank", name="op")
                for hh in range(HPC):
                    nc.tensor.matmul(
                        op[:, :],
                        wo_sb[:, hh, ot * 128:(ot + 1) * 128],
                        mst[:, hh, :],
                        start=(hh == 0), stop=(hh == HPC - 1))
                if ot % 4 == 0:
                    oc_state["oc"] = oc_pool.tile([128, 4, IC], F16,
                                                  tag="oc", name="oc")
                oc = oc_state["oc"]
                if ot % 2 == 1 or oc_state.get("act_only"):
                    nc.scalar.copy(oc[:, ot % 4, :], op[:, :])
                else:
                    nc.vector.tensor_copy(oc[:, ot % 4, :], op[:, :])
                if ot % 4 == 3:
                    # sync only: a gpsimd SWDGE here would delay the
                    # latency-critical affine_select masks
                    dst = out[(ot - 3) * 128:(ot + 1) * 128, gio:gio + IC]
                    nc.sync.dma_start(
                        out=dst.rearrange("(t p) f -> p t f", p=128),
                        in_=oc[:, :, :])

            pending = None
            for g in range(NCH):
                b, ic = divmod(g, NCH // B)
                gio = g * IC
                # ---------------- phase 1: q/k/v for chunk g ----------------
                if g + 1 < NCH:
                    xchs[g + 1] = xpool.tile([128, HT, IC], F16,
                                             tag="xch", name="xch")
                    dma_split(xchs[g + 1], xt[:, (g + 1) * IC:(g + 2) * IC],
                              pieces=(4 if g == 0 else 2))
                xch = xchs.pop(g)
                qt_g = qt_pool.tile([128, HPC, IC], F16, tag="qt", name="qt")
                for wsb, dst in ((wq_sb, None), (wk_sb, kt_sb)):
                    for ot in range(HPC):
                        ps = psum.tile([128, IC], F32, tag="bank", name="ps")
                        for ht in range(HT):
                            nc.tensor.matmul(
                                ps[:, :],
                                wsb[:, ht, ot * 128:(ot + 1) * 128],
                                xch[:, ht, :],
                                start=(ht == 0), stop=(ht == HT - 1))
                        dst_ap = (qt_g[:, ot, :] if dst is None
                                  else dst[:, ot, gio:gio + IC])
                        # q/k copies stay off the laggy DVE queue: the
                        # same-chunk score matmuls depend on them
                        nc.scalar.copy(dst_ap, ps[:, :])
                # ---------------- phase 2: attention for chunk g ------------
                njt = 4 * ic + 4
                # full-width tile first (initializes acc/mx); diagonal
                # tiles (whose exp->mask chain is longest) interleaved
                # among full-width tiles to hide the gpsimd mask latency
                diags = list(range(max(1, 4 * ic), 4 * ic + 4))
                fulls = list(range(1, 4 * ic))
                # scores: diagonals early (their exp->mask chain is the
                # longest); attnV: each diagonal delayed one more slot so
                # the gpsimd mask has ~2 extra PE iterations of cover
                order = [0]
                for i, d in enumerate(diags):
                    order.append(d)
                    if i < len(fulls):
                        order.append(fulls[i])
                order += fulls[len(diags):]
                av_order = [0]
                for i, d in enumerate(diags):
                    if i < len(fulls):
                        av_order.append(fulls[i])
                    av_order.append(d)
                av_order += fulls[len(diags):]

                exds = [{}, {}]

                def emit_sc_exp(h, jt):
                    f0 = max(0, 128 * jt - IC * ic)
                    w = IC - f0
                    sc = psum.tile([128, IC], F32, tag="bank", name="sc")
                    nc.tensor.matmul(
                        sc[:, f0:],
                        kt_sb[:, h, b * L + jt * 128:
                              b * L + (jt + 1) * 128],
                        qt_g[:, h, f0:],
                        start=True, stop=True)
                    ex = exp_pool.tile([128, IC], F16, tag="ex")
                    nc.scalar.activation(
                        ex[:, f0:], sc[:, f0:],
                        mybir.ActivationFunctionType.Exp,
                        scale=SCALE)
                    if jt >= 4 * ic:
                        # zero where local j-partition > local i-column.
                        # gpsimd affine_select: slower as an op (~1.1us)
                        # than a DVE tri-multiply (~0.25us), but the DVE
                        # queue lag makes gpsimd faster end-to-end; DVE
                        # only for chunk 0 while gpsimd generates the
                        # startup SWDGE descriptors.
                        if g == 0:
                            nc.vector.tensor_mul(
                                ex[:, f0:], ex[:, f0:], tri_sb[:, :w])
                        else:
                            nc.gpsimd.affine_select(
                                ex[:, f0:], ex[:, f0:],
                                pattern=[[1, w]],
                                compare_op=mybir.AluOpType.is_ge,
                                fill=0.0, base=0, channel_multiplier=-1)
                    exds[h][jt] = (ex, f0)

                # h0 warmup before the v-projection chains: exp gets a
                # whole projection pass of ACT lead time
                for w0 in range(min(3, njt)):
                    emit_sc_exp(0, order[w0])

                # v in natural (j x o) layout, two 128-token chains per bank
                for half in range(2):
                    vps = psum.tile([128, 2, OPC], F32, tag="bank", name="ps")
                    for sub in range(2):
                        it = half * 2 + sub
                        for ht in range(HT):
                            nc.tensor.matmul(
                                vps[:, sub, :],
                                xch[:, ht, it * 128:(it + 1) * 128],
                                wv_sb[:, ht, :],
                                start=(ht == 0), stop=(ht == HT - 1))
                    vdst = v_sb[:, g * 4 + half * 2:g * 4 + half * 2 + 2, :]
                    # v feeds this chunk's diagonal attnVs — keep both
                    # halves off the laggy DVE queue
                    nc.scalar.copy(vdst, vps[:, :, :])
                filler = []
                if pending is not None:
                    pmst, pgio = pending
                    filler = [(pmst, pgio, ot) for ot in range(H // 128)]
                fill_idx = 0
                # keep the closing DVE chain of the last chunk unqueued
                oc_state["act_only"] = (g == NCH - 1)
                total_iters = HPC * njt
                it_counter = 0
                for h in range(HPC):
                    mx = mx_psum.tile([128, IC], F32, tag="mx", name="mx")
                    # fp16 accumulator: feeds the PE rowsum directly
                    # (no cast), DVE 16-bit ops run at the 2x rate
                    acc = acc_pool.tile([128, IC], F16, tag="acc", name="acc")
                    exd = exds[h]

                    if h == 1:
                        for w0 in range(min(3, njt)):
                            emit_sc_exp(h, order[w0])
                    # last two tiles skip the DVE acc and feed the PE
                    # rowsum directly, so the rowsum never waits on DVE
                    direct = []
                    for idx in range(njt):
                        jt = av_order[idx]
                        it_counter += 1
                        if idx + 3 < njt:
                            emit_sc_exp(h, order[idx + 3])
                        ex, f0 = exd.pop(jt)
                        nc.tensor.matmul(
                            mx[:, f0:],
                            v_sb[:, b * JTB + jt, h * 128:(h + 1) * 128],
                            ex[:, f0:],
                            start=(idx == 0), stop=(idx == njt - 1))
                        if idx == 0:
                            nc.vector.tensor_copy(acc[:, :], ex[:, :])
                        elif idx < njt - 2:
                            nc.vector.tensor_add(
                                acc[:, f0:], acc[:, f0:], ex[:, f0:])
                        else:
                            direct.append((ex, f0))
                        # hold 2 fillers back so each h's rowsum below
                        # always has PE cover for the last DVE add
                        want = min((it_counter * len(filler)) // total_iters,
                                   max(0, len(filler) - 2))
                        while fill_idx < want:
                            emit_oproj_ot(*filler[fill_idx])
                            fill_idx += 1
                    # fillers before the rowsum cover the last DVE add's
                    # queue lag
                    for _ in range(2):
                        if fill_idx < len(filler):
                            emit_oproj_ot(*filler[fill_idx])
                            fill_idx += 1
                    # inline rowsum: acc + the two direct ex tiles
                    rs = psum.tile([128, IC], F32, tag="bank", name="rs")
                    nc.tensor.matmul(rs[0:1, :], ones_sb[:, :], acc[:, :],
                                     start=True, stop=False)
                    for di, (ex, f0) in enumerate(direct):
                        nc.tensor.matmul(
                            rs[0:1, f0:], ones_sb[:, :], ex[:, f0:],
                            start=False, stop=(di == len(direct) - 1))
                    # ACT row-copy releases the rs bank fast (DVE recip
                    # would hold it ~2us and starve the sc ring)
                    tmp = rr_pool.tile([128, IC], F32, tag="red", name="tmp")
                    nc.scalar.copy(tmp[0:1, :], rs[0:1, :])
                    rec = rr_pool.tile([128, IC], F32, tag="red", name="rec")
                    nc.vector.reciprocal_approx_fast(
                        out=rec[0:1, :], in_=tmp[0:1, :])
                    nc.gpsimd.partition_broadcast(
                        rec[:, :], rec[0:1, :], channels=128)
                    mst = (mst_pool.tile([128, HPC, IC], F16, tag="mst",
                                         name="mst")
                           if h == 0 else mst)
                    nc.vector.tensor_mul(mst[:, h, :], mx[:, :], rec[:, :])
                while fill_idx < len(filler):
                    emit_oproj_ot(*filler[fill_idx])
                    fill_idx += 1
                pending = (mst, gio)
            # final flush: 2-ot DMA groups rotated across four queues so
            # the drain overlaps instead of serializing on one queue
            pmst, pgio = pending
            for ot in range(H // 128):
                op = psum.tile([128, IC], F32, tag="bank", name="op")
                for hh in range(HPC):
                    nc.tensor.matmul(
                        op[:, :],
                        wo_sb[:, hh, ot * 128:(ot + 1) * 128],
                        pmst[:, hh, :],
                        start=(hh == 0), stop=(hh == HPC - 1))
                if ot % 2 == 0:
                    oc_state["oc"] = oc_pool.tile([128, 2, IC], F16,
                                                  tag="oc2", name="oc2")
                oc = oc_state["oc"]
                if ot % 2 == 1:
                    nc.scalar.copy(oc[:, ot % 2, :], op[:, :])
                else:
                    nc.vector.tensor_copy(oc[:, ot % 2, :], op[:, :])
                if ot % 2 == 1:
                    deng = (nc.sync, nc.scalar, nc.gpsimd)[(ot // 2) % 3]
                    dst = out[(ot - 1) * 128:(ot + 1) * 128,
                              pgio:pgio + IC]
                    deng.dma_start(
                        out=dst.rearrange("(t p) f -> p t f", p=128),
                        in_=oc[:, :, :])
    nc.finalize()
    return nc


_NC_CACHE = None


def _get_nc():
    global _NC_CACHE
    if _NC_CACHE is None:
        _NC_CACHE = _build()
    return _NC_CACHE


def _install_hook_shim():
    """Make antenv.axon_hooks importable (absent on this image) so
    run_bass_kernel_spmd's trace path degrades gracefully."""
    import types
    import antenv
    if "antenv.axon_hooks" not in sys.modules:
        shim = types.ModuleType("antenv.axon_hooks")

        def set_axon_ntff_profile_hook(h):
            shim._the_hook = h

        def get_axon_ntff_profile_hook():
            return getattr(shim, "_the_hook", None)

        shim.set_axon_ntff_profile_hook = set_axon_ntff_profile_hook
        shim.get_axon_ntff_profile_hook = get_axon_ntff_profile_hook
        sys.modules["antenv.axon_hooks"] = shim
        antenv.axon_hooks = shim


def _enable_profiling():
    """Wire the axon NTFF profile hook for neuron-profile timing."""
    _install_hook_shim()
    from trn_agent_boot.trn_boot import _ntff_profile_via_ctypes
    hook = _ntff_profile_via_ctypes("/opt/axon/libaxon_pjrt.so")
    sys.modules["antenv.axon_hooks"].set_axon_ntff_profile_hook(hook)
    import concourse.bass_utils as bu
    bu.upload_artifacts = lambda tmpdir: "local://" + tmpdir


def kernel(x, padding_mask, Wq, Wk, Wv, Wo):
    global LAST_EXEC_NS
    f16 = np.float16
    x = np.asarray(x, dtype=np.float32)

    xt = np.ascontiguousarray(x.reshape(BL, H).T).astype(f16)   # (H, BL)
    wqt = np.asarray(Wq, dtype=np.float32).T.astype(f16)        # [h, o]
    wkt = np.asarray(Wk, dtype=np.float32).T.astype(f16)
    wvt = np.asarray(Wv, dtype=np.float32).T.astype(f16)
    wot = np.asarray(Wo, dtype=np.float32).T.astype(f16)        # [h_in, o]

    tri = np.triu(np.ones((128, IC), dtype=f16))

    in_maps = []
    for c in range(N_CORES):
        sl = slice(c * OPC, (c + 1) * OPC)
        in_maps.append({
            "xt": xt,
            "wq": np.ascontiguousarray(wqt[:, sl]),
            "wk": np.ascontiguousarray(wkt[:, sl]),
            "wv": np.ascontiguousarray(wvt[:, sl]),
            "wo": np.ascontiguousarray(wot[sl, :]),
            "tri": tri,
        })

    profile = os.environ.get("KERNEL_PROFILE", "0") == "1"
    try:
        if profile:
            _enable_profiling()
        else:
            _install_hook_shim()
    except Exception:
        profile = False

    nc = _get_nc()
    res = run_bass_kernel_spmd(nc, in_maps, core_ids=list(range(N_CORES)),
                               trace=profile)
    LAST_EXEC_NS = res.exec_time_ns

    total = np.zeros((H, BL), dtype=np.float32)
    for c in range(N_CORES):
        total += np.asarray(res.results[c]["out"], dtype=np.float32)
    return np.ascontiguousarray(total.T).astype(np.float32).reshape(B, L, H)
